# revision 25
# baseline (speedup 1.0000x reference)
"""Trainium2 Bass kernel for nn_AutoencoderHybrid_65481071408310.

Math: the reference simulates an 8-qubit circuit per sample. The RX-encoding
layer produces a product state whose amplitudes factor as
    psi[k] = m[k] * (-i)^popcount(k),   m[k] = prod_i (cos(x_i/2) or sin(x_i/2))
and the StronglyEntanglingLayers form a fixed 256x256 unitary U that depends
only on q_weights.  Folding the popcount phases into U gives a REAL matmul
    phi = m @ V,  V = [Re(W) | Im(W)],  W = (U * (-i)^popcount)^T   (256 x 512)
then probs = phi_r^2 + phi_i^2, z_i = probs @ signs, and the MLP head.
signs@w1.T folds into A (256x4); stacking A2=[A;A] lets the squared 512-wide
phi contract directly (no pairwise adds).

Device pipeline per core (batch 8192, fp16 matmul operands):
  ACT: cos/sin; PE: transpose to (wire, sample) layout; replication DMAs +
  DVE/GPSIMD fp16 muls build the outer-product mT (256 x samples) in
  transposed layout; PE: K=256 matmul -> phi (512 wide), squares (ACT+DVE),
  PE: A2 contraction (K=512 -> 4), relu (+b1) on ACT, PE: w2 head (+b2 on
  copy-out), strided DMA to (B, 8).
"""
import sys
import numpy as np

sys.path.insert(0, '/opt/trn_rl_repo')

import concourse.bacc as bacc
import concourse.mybir as mybir
import concourse.tile as tile
from concourse.bass_utils import run_bass_kernel_spmd

F32 = mybir.dt.float32
F16 = mybir.dt.float16
AFT = mybir.ActivationFunctionType
ALU = mybir.AluOpType

NQ = 8
DIM = 256
REPS = 4
INPUT_DIM = 8
LATENT = 4
BATCH = 65536
NCORES = 8
BC = BATCH // NCORES          # 8192 samples per core
NCHUNK = BC // 128            # 64 chunks of 128 samples
NCTILE = NCHUNK // 16         # 4 ctile groups (16 chunks each)
CF = 16 * 128                 # 2048 free elems per ctile
NBLK = BC // 512              # 16 blocks of 512 samples
BPC = 4                       # blocks per ctile

LAST_RESULTS = None           # test harness introspection


# ---------------------------------------------------------------- host math
def _rot_mat(phi, theta, omega):
    c, s = np.cos(theta / 2), np.sin(theta / 2)
    return np.array([
        [np.exp(-0.5j * (phi + omega)) * c, -np.exp(0.5j * (phi - omega)) * s],
        [np.exp(-0.5j * (phi - omega)) * s, np.exp(0.5j * (phi + omega)) * c],
    ], dtype=np.complex128)


def _kron_list(ops):
    full = ops[0]
    for o in ops[1:]:
        full = np.kron(full, o)
    return full


def _build_entangler(qw):
    I2 = np.eye(2, dtype=np.complex128)
    P0 = np.array([[1, 0], [0, 0]], dtype=np.complex128)
    P1 = np.array([[0, 0], [0, 1]], dtype=np.complex128)
    X = np.array([[0, 1], [1, 0]], dtype=np.complex128)
    U = np.eye(DIM, dtype=np.complex128)
    for l in range(REPS):
        for i in range(NQ):
            ops = [I2] * NQ
            ops[i] = _rot_mat(*qw[l, i])
            U = _kron_list(ops) @ U
        r = (l % (NQ - 1)) + 1
        for i in range(NQ):
            t = (i + r) % NQ
            ops0 = [I2] * NQ
            ops0[i] = P0
            ops1 = [I2] * NQ
            ops1[i] = P1
            ops1[t] = X
            U = (_kron_list(ops0) + _kron_list(ops1)) @ U
    return U


def _host_consts(q_weights, w1, b1, w2, b2):
    U = _build_entangler(q_weights.astype(np.float64))
    pop = np.array([bin(k).count('1') for k in range(DIM)])
    W = (U * ((-1j) ** pop)[None, :]).T          # phi = m @ W
    V = np.concatenate([W.real, W.imag], axis=1)  # (256, 512)
    ks = np.arange(DIM)
    signs = 1.0 - 2.0 * ((ks[:, None] >> (NQ - 1 - np.arange(NQ))[None, :]) & 1)
    A = signs @ w1.T.astype(np.float64)           # (256, 4)
    vmat = np.ascontiguousarray(
        V.reshape(2, 128, 512).transpose(1, 0, 2).reshape(128, 1024)
        .astype(np.float16))
    amat = np.ascontiguousarray(
        A.reshape(2, 128, LATENT).transpose(1, 0, 2).reshape(128, 2 * LATENT)
        .astype(np.float16))
    w2b = np.concatenate([w2.T.astype(np.float64),
                          b2.astype(np.float64)[None, :]], axis=0)  # (5, 8)
    return {
        'vmat': vmat,
        'amat': amat,
        'w2b': np.ascontiguousarray(w2b.astype(np.float16)),
        'b1c': np.ascontiguousarray(b1.astype(np.float32).reshape(LATENT, 1)),
        'ident': np.eye(128, dtype=np.float16),
    }


# ---------------------------------------------------------------- bass build
def _build_nc():
    nc = bacc.Bacc(None, target_bir_lowering=False)
    xs = nc.declare_dram_parameter("xs", [BC, INPUT_DIM], F32, isOutput=False)
    vmat = nc.declare_dram_parameter("vmat", [128, 1024], F16, isOutput=False)
    amat = nc.declare_dram_parameter("amat", [128, 2 * LATENT], F16, isOutput=False)
    w2b = nc.declare_dram_parameter("w2b", [LATENT + 1, INPUT_DIM], F16, isOutput=False)
    b1c = nc.declare_dram_parameter("b1c", [LATENT, 1], F32, isOutput=False)
    ident = nc.declare_dram_parameter("ident", [128, 128], F16, isOutput=False)
    out = nc.declare_dram_parameter("out", [BC, INPUT_DIM], F32, isOutput=True)

    CH = 4096              # free elems per half (32 chunks)

    with tile.TileContext(nc) as tc:
        with (
            tc.tile_pool(name="const", bufs=1) as cst,
            tc.tile_pool(name="cs", bufs=1) as csp,
            tc.tile_pool(name="stage", bufs=1) as stg,
            tc.tile_pool(name="mtp", bufs=2) as mtp,
            tc.tile_pool(name="blk", bufs=6) as blk,
            tc.tile_pool(name="small", bufs=2) as sml,
            tc.tile_pool(name="tps", bufs=1, space="PSUM") as tpsp,
            tc.tile_pool(name="phip", bufs=2, space="PSUM") as phip,
            tc.tile_pool(name="prehp", bufs=2, space="PSUM") as prehp,
            tc.tile_pool(name="woutp", bufs=1, space="PSUM") as woutp,
        ):
            # ---- constants
            vt = cst.tile([128, 1024], F16)
            nc.sync.dma_start(vt[:], vmat[:])
            at = cst.tile([128, 2 * LATENT], F16)
            nc.sync.dma_start(at[:], amat[:])
            w2s = cst.tile([LATENT + 1, INPUT_DIM], F16)
            nc.sync.dma_start(w2s[:], w2b[:])
            b1s = cst.tile([LATENT, 1], F32)
            nc.sync.dma_start(b1s[:], b1c[:])
            ids = cst.tile([128, 128], F16)
            nc.sync.dma_start(ids[:], ident[:])
            halfpi = cst.tile([128, 1], F32)
            nc.vector.memset(halfpi[:], float(np.pi / 2))
            zero = cst.tile([128, 1], F32)
            nc.vector.memset(zero[:], 0.0)

            # ---- whole-core cos/sin, natural layout; sample = 64p + n
            xnat = csp.tile([128, BC // 16], F32)      # free = (n, d)
            nc.sync.dma_start(xnat[:], xs.rearrange("(p n) d -> p n d", n=64))
            cnat = csp.tile([128, BC // 16], F16)
            snat = csp.tile([128, BC // 16], F16)
            xdn = xnat.rearrange("p (n d) -> p d n", d=8)
            nc.scalar.activation(cnat.rearrange("p (d n) -> p d n", d=8),
                                 xdn, AFT.Sin, scale=0.5, bias=halfpi[:])
            nc.scalar.activation(snat.rearrange("p (d n) -> p d n", d=8),
                                 xdn, AFT.Sin, scale=0.5, bias=zero[:])

            # ---- all 8 transposes upfront into per-ctile (wire, sample) tiles
            # cnat free = (d, n): slice u holds wires {2u, 2u+1} x n in [0,64)
            # cTs[u]: row 64*(w%2)+n = wire w=2u+(w%2), chunk n
            cTs, sTs = [], []
            for u in range(4):
                ctp = tpsp.tile([128, 128], F16, tag="tp")
                nc.tensor.transpose(ctp[:], cnat[:, 128 * u:128 * (u + 1)], ids[:])
                cTu = csp.tile([128, 128], F16, tag=f"cT{u}")
                nc.vector.tensor_copy(cTu[:], ctp[:])
                cTs.append(cTu)
                stp = tpsp.tile([128, 128], F16, tag="tp")
                nc.tensor.transpose(stp[:], snat[:, 128 * u:128 * (u + 1)], ids[:])
                sTu = csp.tile([128, 128], F16, tag=f"sT{u}")
                nc.vector.tensor_copy(sTu[:], stp[:])
                sTs.append(sTu)

            def stage_q(c0, nch):
                CH = 128 * nch
                csf = stg.tile([16, CH], F16, tag="csf")
                for w in range(8):
                    rows = slice(64 * (w % 2) + c0, 64 * (w % 2) + c0 + nch)
                    nc.sync.dma_start(csf[w:w + 1, :], cTs[w // 2][rows, :])
                    nc.scalar.dma_start(csf[8 + w:9 + w, :], sTs[w // 2][rows, :])

                pairsA = stg.tile([16, CH], F16, tag="pairsA")
                pairsB = stg.tile([16, CH], F16, tag="pairsB")
                for q in range(4):
                    nc.gpsimd.dma_start(
                        pairsA[4 * q:4 * q + 4, :],
                        csf[2 * q::8, :].unsqueeze(1).broadcast_to([2, 2, CH]))
                    nc.sync.dma_start(pairsB[4 * q:4 * q + 2, :],
                                      csf[2 * q + 1::8, :])
                    nc.sync.dma_start(pairsB[4 * q + 2:4 * q + 4, :],
                                      csf[2 * q + 1::8, :])
                pairs = stg.tile([16, CH], F16, tag="pairs")
                nc.vector.tensor_mul(pairs[:], pairsA[:], pairsB[:])

                hiloA = stg.tile([32, CH], F16, tag="hiloA")
                hiloB = stg.tile([32, CH], F16, tag="hiloB")
                nc.gpsimd.dma_start(
                    hiloA[0:16], pairs[0:4].unsqueeze(1).broadcast_to([4, 4, CH]))
                nc.gpsimd.dma_start(
                    hiloA[16:32], pairs[8:12].unsqueeze(1).broadcast_to([4, 4, CH]))
                for k in range(4):
                    nc.sync.dma_start(hiloB[4 * k:4 * k + 4], pairs[4:8])
                    nc.sync.dma_start(hiloB[16 + 4 * k:20 + 4 * k], pairs[12:16])
                hilo = stg.tile([32, CH], F16, tag="hilo")
                nc.vector.tensor_mul(hilo[:], hiloA[:], hiloB[:])

                mtA0 = stg.tile([128, CH], F16, tag="mtA0")
                mtA1 = stg.tile([128, CH], F16, tag="mtA1")
                mtB = stg.tile([128, CH], F16, tag="mtB")
                h4 = stg.tile([96, CH], F16, tag="h4")
                nc.gpsimd.dma_start(
                    h4[0:32], hilo[0:8].unsqueeze(1).broadcast_to([8, 4, CH]))
                nc.gpsimd.dma_start(
                    h4[32:64], hilo[8:16].unsqueeze(1).broadcast_to([8, 4, CH]))
                nc.sync.dma_start(h4[64:80], hilo[16:32])
                nc.sync.dma_start(h4[80:96], hilo[16:32])
                nc.gpsimd.dma_start(
                    mtA0[:], h4[0:32].unsqueeze(1).broadcast_to([32, 4, CH]))
                nc.gpsimd.dma_start(
                    mtA1[:], h4[32:64].unsqueeze(1).broadcast_to([32, 4, CH]))
                nc.gpsimd.dma_start(mtB[0:32], h4[64:96])
                nc.gpsimd.dma_start(mtB[32:64], h4[64:96])
                nc.sync.dma_start(mtB[64:96], h4[64:96])
                nc.sync.dma_start(mtB[96:128], h4[64:96])
                mt0 = mtp.tile([128, CH], F16, tag="mt0")
                mt1 = mtp.tile([128, CH], F16, tag="mt1")
                nc.vector.tensor_mul(mt0[:], mtA0[:], mtB[:])
                nc.vector.tensor_mul(mt1[:], mtA1[:], mtB[:])
                return mt0, mt1

            def compute_q(c0, nch, mt0, mt1):
                nblk = nch // 4
                onat = sml.tile([128, 8 * nch], F32, tag="onat")
                for gg in range(nblk):
                    sl = slice(512 * gg, 512 * (gg + 1))
                    probs = []
                    for jp in range(2):
                        phi = phip.tile([128, 1024], F32, tag="phi")
                        for e in range(2):
                            jt = 2 * jp + e
                            nc.tensor.matmul(
                                phi[:, 512 * e:512 * (e + 1)],
                                vt[:, 128 * jt:128 * (jt + 1)],
                                mt0[:, sl], start=True, stop=False)
                            nc.tensor.matmul(
                                phi[:, 512 * e:512 * (e + 1)],
                                vt[:, 512 + 128 * jt:512 + 128 * (jt + 1)],
                                mt1[:, sl], start=False, stop=True)
                        pr = blk.tile([128, 1024], F16, tag="probs")
                        nc.scalar.activation(pr[:], phi[:], AFT.Square,
                                             bias=zero[:])
                        probs.append(pr)
                    preh = prehp.tile([LATENT, 512], F32, tag="preh")
                    for jt in range(4):
                        ab = at[:, 4 * (jt % 2):4 * (jt % 2) + 4]
                        nc.tensor.matmul(preh[:],
                                         ab, probs[jt // 2][:, 512 * (jt % 2):
                                                            512 * (jt % 2) + 512],
                                         start=(jt == 0), stop=(jt == 3))
                    h5 = sml.tile([LATENT + 1, 512], F16, tag="h5")
                    nc.gpsimd.memset(h5[:], 1.0)
                    nc.vector.tensor_scalar(h5[0:LATENT, :], preh[:],
                                            b1s[:], 0.0,
                                            mybir.AluOpType.add,
                                            mybir.AluOpType.max)
                    wnat = woutp.tile([128, 4 * INPUT_DIM], F32, tag="wnat")
                    for c in range(4):
                        nc.tensor.matmul(
                            wnat[:, 8 * c:8 * (c + 1)],
                            h5[:, 128 * c:128 * (c + 1)], w2s[:],
                            start=True, stop=True)
                    nc.vector.tensor_copy(
                        onat[:, 32 * gg:32 * (gg + 1)], wnat[:])
                nc.scalar.dma_start(
                    out.rearrange("(p n) d -> p n d", n=64)[:, c0:c0 + nch, :],
                    onat[:])

            PHASES = [(0, 16), (16, 16), (32, 16), (48, 16)]
            mts = [stage_q(*PHASES[0]), stage_q(*PHASES[1])]
            for i, ph in enumerate(PHASES):
                if i + 2 < len(PHASES):
                    mts.append(stage_q(*PHASES[i + 2]))
                compute_q(*ph, *mts[i])

    nc.compile()
    return nc


_NC_CACHE = []


def _get_nc():
    if not _NC_CACHE:
        _NC_CACHE.append(_build_nc())
    return _NC_CACHE[0]


def kernel(x, q_weights, w1, b1, w2, b2):
    global LAST_RESULTS
    x = np.ascontiguousarray(np.asarray(x, dtype=np.float32))
    consts = _host_consts(np.asarray(q_weights), np.asarray(w1),
                          np.asarray(b1), np.asarray(w2), np.asarray(b2))
    nc = _get_nc()
    in_maps = [
        {'xs': np.ascontiguousarray(x[i * BC:(i + 1) * BC]), **consts}
        for i in range(NCORES)
    ]
    res = run_bass_kernel_spmd(nc, in_maps, list(range(NCORES)))
    LAST_RESULTS = res
    return np.concatenate([res.results[i]['out'] for i in range(NCORES)],
                          axis=0).astype(np.float32)


# revision 26
# speedup vs baseline: 1.1293x; 1.1293x over previous
"""Trainium2 Bass kernel for nn_AutoencoderHybrid_65481071408310.

Math: the reference simulates an 8-qubit circuit per sample. The RX-encoding
layer produces a product state whose amplitudes factor as
    psi[k] = m[k] * (-i)^popcount(k),   m[k] = prod_i (cos(x_i/2) or sin(x_i/2))
and the StronglyEntanglingLayers form a fixed 256x256 unitary U that depends
only on q_weights.  Folding the popcount phases into U gives a REAL matmul
    phi = m @ V,  V = [Re(W) | Im(W)],  W = (U * (-i)^popcount)^T   (256 x 512)
then probs = phi_r^2 + phi_i^2, z_i = probs @ signs, and the MLP head.
signs@w1.T folds into A (256x4); stacking A2=[A;A] lets the squared 512-wide
phi contract directly (no pairwise adds).

Device pipeline per core (batch 8192, fp16 matmul operands):
  ACT: cos/sin; PE: transpose to (wire, sample) layout; replication DMAs +
  DVE/GPSIMD fp16 muls build the outer-product mT (256 x samples) in
  transposed layout; PE: K=256 matmul -> phi (512 wide), squares (ACT+DVE),
  PE: A2 contraction (K=512 -> 4), relu (+b1) on ACT, PE: w2 head (+b2 on
  copy-out), strided DMA to (B, 8).
"""
import sys
import numpy as np

sys.path.insert(0, '/opt/trn_rl_repo')

import concourse.bacc as bacc
import concourse.mybir as mybir
import concourse.tile as tile
from concourse.bass_utils import run_bass_kernel_spmd

F32 = mybir.dt.float32
F16 = mybir.dt.float16
AFT = mybir.ActivationFunctionType
ALU = mybir.AluOpType

NQ = 8
DIM = 256
REPS = 4
INPUT_DIM = 8
LATENT = 4
BATCH = 65536
NCORES = 8
BC = BATCH // NCORES          # 8192 samples per core
NCHUNK = BC // 128            # 64 chunks of 128 samples
NCTILE = NCHUNK // 16         # 4 ctile groups (16 chunks each)
CF = 16 * 128                 # 2048 free elems per ctile
NBLK = BC // 512              # 16 blocks of 512 samples
BPC = 4                       # blocks per ctile

LAST_RESULTS = None           # test harness introspection


# ---------------------------------------------------------------- host math
def _rot_mat(phi, theta, omega):
    c, s = np.cos(theta / 2), np.sin(theta / 2)
    return np.array([
        [np.exp(-0.5j * (phi + omega)) * c, -np.exp(0.5j * (phi - omega)) * s],
        [np.exp(-0.5j * (phi - omega)) * s, np.exp(0.5j * (phi + omega)) * c],
    ], dtype=np.complex128)


def _kron_list(ops):
    full = ops[0]
    for o in ops[1:]:
        full = np.kron(full, o)
    return full


def _build_entangler(qw):
    I2 = np.eye(2, dtype=np.complex128)
    P0 = np.array([[1, 0], [0, 0]], dtype=np.complex128)
    P1 = np.array([[0, 0], [0, 1]], dtype=np.complex128)
    X = np.array([[0, 1], [1, 0]], dtype=np.complex128)
    U = np.eye(DIM, dtype=np.complex128)
    for l in range(REPS):
        for i in range(NQ):
            ops = [I2] * NQ
            ops[i] = _rot_mat(*qw[l, i])
            U = _kron_list(ops) @ U
        r = (l % (NQ - 1)) + 1
        for i in range(NQ):
            t = (i + r) % NQ
            ops0 = [I2] * NQ
            ops0[i] = P0
            ops1 = [I2] * NQ
            ops1[i] = P1
            ops1[t] = X
            U = (_kron_list(ops0) + _kron_list(ops1)) @ U
    return U


def _host_consts(q_weights, w1, b1, w2, b2):
    U = _build_entangler(q_weights.astype(np.float64))
    pop = np.array([bin(k).count('1') for k in range(DIM)])
    W = (U * ((-1j) ** pop)[None, :]).T          # phi = m @ W
    V = np.concatenate([W.real, W.imag], axis=1)  # (256, 512)
    ks = np.arange(DIM)
    signs = 1.0 - 2.0 * ((ks[:, None] >> (NQ - 1 - np.arange(NQ))[None, :]) & 1)
    A = signs @ w1.T.astype(np.float64)           # (256, 4)
    vmat = np.ascontiguousarray(
        V.reshape(2, 128, 512).transpose(1, 0, 2).reshape(128, 1024)
        .astype(np.float16))
    amat = np.ascontiguousarray(
        A.reshape(2, 128, LATENT).transpose(1, 0, 2).reshape(128, 2 * LATENT)
        .astype(np.float16))
    w2b = np.concatenate([w2.T.astype(np.float64),
                          b2.astype(np.float64)[None, :]], axis=0)  # (5, 8)
    return {
        'vmat': vmat,
        'amat': amat,
        'w2b': np.ascontiguousarray(w2b.astype(np.float16)),
        'b1c': np.ascontiguousarray(b1.astype(np.float32).reshape(LATENT, 1)),
        'ident': np.eye(128, dtype=np.float16),
    }


# ---------------------------------------------------------------- bass build
def _build_nc():
    nc = bacc.Bacc(None, target_bir_lowering=False)
    xs = nc.declare_dram_parameter("xs", [BC, INPUT_DIM], F32, isOutput=False)
    vmat = nc.declare_dram_parameter("vmat", [128, 1024], F16, isOutput=False)
    amat = nc.declare_dram_parameter("amat", [128, 2 * LATENT], F16, isOutput=False)
    w2b = nc.declare_dram_parameter("w2b", [LATENT + 1, INPUT_DIM], F16, isOutput=False)
    b1c = nc.declare_dram_parameter("b1c", [LATENT, 1], F32, isOutput=False)
    ident = nc.declare_dram_parameter("ident", [128, 128], F16, isOutput=False)
    out = nc.declare_dram_parameter("out", [BC, INPUT_DIM], F32, isOutput=True)

    CH = 4096              # free elems per half (32 chunks)

    with tile.TileContext(nc) as tc:
        with (
            tc.tile_pool(name="const", bufs=1) as cst,
            tc.tile_pool(name="cs", bufs=1) as csp,
            tc.tile_pool(name="stage", bufs=1) as stg,
            tc.tile_pool(name="mtp", bufs=2) as mtp,
            tc.tile_pool(name="blk", bufs=6) as blk,
            tc.tile_pool(name="small", bufs=2) as sml,
            tc.tile_pool(name="tps", bufs=1, space="PSUM") as tpsp,
            tc.tile_pool(name="phip", bufs=2, space="PSUM") as phip,
            tc.tile_pool(name="prehp", bufs=2, space="PSUM") as prehp,
            tc.tile_pool(name="woutp", bufs=1, space="PSUM") as woutp,
        ):
            # ---- constants
            vt = cst.tile([128, 1024], F16)
            nc.sync.dma_start(vt[:], vmat[:])
            at = cst.tile([128, 2 * LATENT], F16)
            nc.sync.dma_start(at[:], amat[:])
            w2s = cst.tile([LATENT + 1, INPUT_DIM], F16)
            nc.sync.dma_start(w2s[:], w2b[:])
            b1s = cst.tile([LATENT, 1], F32)
            nc.sync.dma_start(b1s[:], b1c[:])
            ids = cst.tile([128, 128], F16)
            nc.sync.dma_start(ids[:], ident[:])
            halfpi = cst.tile([128, 1], F32)
            nc.vector.memset(halfpi[:], float(np.pi / 2))
            zero = cst.tile([128, 1], F32)
            nc.vector.memset(zero[:], 0.0)

            # ---- whole-core cos/sin, natural layout; sample = 64p + n
            xnat = csp.tile([128, BC // 16], F32)      # free = (n, d)
            nc.sync.dma_start(xnat[:], xs.rearrange("(p n) d -> p n d", n=64))
            cnat = csp.tile([128, BC // 16], F16)
            snat = csp.tile([128, BC // 16], F16)
            xdn = xnat.rearrange("p (n d) -> p d n", d=8)
            nc.scalar.activation(cnat.rearrange("p (d n) -> p d n", d=8),
                                 xdn, AFT.Sin, scale=0.5, bias=halfpi[:])
            nc.scalar.activation(snat.rearrange("p (d n) -> p d n", d=8),
                                 xdn, AFT.Sin, scale=0.5, bias=zero[:])

            # ---- all 8 transposes upfront into per-ctile (wire, sample) tiles
            # cnat free = (d, n): slice u holds wires {2u, 2u+1} x n in [0,64)
            # cTs[u]: row 64*(w%2)+n = wire w=2u+(w%2), chunk n
            cTs, sTs = [], []
            for u in range(4):
                ctp = tpsp.tile([128, 128], F16, tag="tp")
                nc.tensor.transpose(ctp[:], cnat[:, 128 * u:128 * (u + 1)], ids[:])
                cTu = csp.tile([128, 128], F16, tag=f"cT{u}")
                nc.vector.tensor_copy(cTu[:], ctp[:])
                cTs.append(cTu)
                stp = tpsp.tile([128, 128], F16, tag="tp")
                nc.tensor.transpose(stp[:], snat[:, 128 * u:128 * (u + 1)], ids[:])
                sTu = csp.tile([128, 128], F16, tag=f"sT{u}")
                nc.vector.tensor_copy(sTu[:], stp[:])
                sTs.append(sTu)

            def stage_q(c0, nch):
                CH = 128 * nch
                csf = stg.tile([16, CH], F16, tag="csf")
                for w in range(8):
                    rows = slice(64 * (w % 2) + c0, 64 * (w % 2) + c0 + nch)
                    nc.sync.dma_start(csf[w:w + 1, :], cTs[w // 2][rows, :])
                    nc.scalar.dma_start(csf[8 + w:9 + w, :], sTs[w // 2][rows, :])

                pairsA = stg.tile([16, CH], F16, tag="pairsA")
                pairsB = stg.tile([16, CH], F16, tag="pairsB")
                for q in range(4):
                    nc.gpsimd.dma_start(
                        pairsA[4 * q:4 * q + 4, :],
                        csf[2 * q::8, :].unsqueeze(1).broadcast_to([2, 2, CH]))
                    nc.sync.dma_start(pairsB[4 * q:4 * q + 2, :],
                                      csf[2 * q + 1::8, :])
                    nc.sync.dma_start(pairsB[4 * q + 2:4 * q + 4, :],
                                      csf[2 * q + 1::8, :])
                pairs = stg.tile([16, CH], F16, tag="pairs")
                nc.vector.tensor_mul(pairs[:], pairsA[:], pairsB[:])

                hiloA = stg.tile([32, CH], F16, tag="hiloA")
                hiloB = stg.tile([32, CH], F16, tag="hiloB")
                nc.gpsimd.dma_start(
                    hiloA[0:16], pairs[0:4].unsqueeze(1).broadcast_to([4, 4, CH]))
                nc.gpsimd.dma_start(
                    hiloA[16:32], pairs[8:12].unsqueeze(1).broadcast_to([4, 4, CH]))
                for k in range(4):
                    nc.sync.dma_start(hiloB[4 * k:4 * k + 4], pairs[4:8])
                    nc.sync.dma_start(hiloB[16 + 4 * k:20 + 4 * k], pairs[12:16])
                hilo = stg.tile([32, CH], F16, tag="hilo")
                nc.vector.tensor_mul(hilo[:], hiloA[:], hiloB[:])

                mtA0 = stg.tile([128, CH], F16, tag="mtA0")
                mtA1 = stg.tile([128, CH], F16, tag="mtA1")
                mtB = stg.tile([128, CH], F16, tag="mtB")
                h4 = stg.tile([96, CH], F16, tag="h4")
                nc.gpsimd.dma_start(
                    h4[0:32], hilo[0:8].unsqueeze(1).broadcast_to([8, 4, CH]))
                nc.gpsimd.dma_start(
                    h4[32:64], hilo[8:16].unsqueeze(1).broadcast_to([8, 4, CH]))
                nc.sync.dma_start(h4[64:80], hilo[16:32])
                nc.sync.dma_start(h4[80:96], hilo[16:32])
                nc.gpsimd.dma_start(
                    mtA0[:], h4[0:32].unsqueeze(1).broadcast_to([32, 4, CH]))
                nc.gpsimd.dma_start(
                    mtA1[:], h4[32:64].unsqueeze(1).broadcast_to([32, 4, CH]))
                nc.gpsimd.dma_start(mtB[0:32], h4[64:96])
                nc.gpsimd.dma_start(mtB[32:64], h4[64:96])
                nc.sync.dma_start(mtB[64:96], h4[64:96])
                nc.sync.dma_start(mtB[96:128], h4[64:96])
                mt0 = mtp.tile([128, CH], F16, tag="mt0")
                mt1 = mtp.tile([128, CH], F16, tag="mt1")
                nc.vector.tensor_mul(mt0[:], mtA0[:], mtB[:])
                nc.vector.tensor_mul(mt1[:], mtA1[:], mtB[:])
                return mt0, mt1

            def compute_q(c0, nch, mt0, mt1):
                nblk = nch // 4
                onat = sml.tile([128, 8 * nch], F32, tag="onat")
                for gg in range(nblk):
                    sl = slice(512 * gg, 512 * (gg + 1))
                    probs = []
                    for jp in range(2):
                        phi = phip.tile([128, 1024], F32, tag="phi")
                        for e in range(2):
                            jt = 2 * jp + e
                            nc.tensor.matmul(
                                phi[:, 512 * e:512 * (e + 1)],
                                vt[:, 128 * jt:128 * (jt + 1)],
                                mt0[:, sl], start=True, stop=False)
                            nc.tensor.matmul(
                                phi[:, 512 * e:512 * (e + 1)],
                                vt[:, 512 + 128 * jt:512 + 128 * (jt + 1)],
                                mt1[:, sl], start=False, stop=True)
                        pr = blk.tile([128, 1024], F16, tag="probs")
                        nc.scalar.activation(pr[:], phi[:], AFT.Square,
                                             bias=zero[:])
                        probs.append(pr)
                    preh = prehp.tile([LATENT, 512], F32, tag="preh")
                    for jt in range(4):
                        ab = at[:, 4 * (jt % 2):4 * (jt % 2) + 4]
                        nc.tensor.matmul(preh[:],
                                         ab, probs[jt // 2][:, 512 * (jt % 2):
                                                            512 * (jt % 2) + 512],
                                         start=(jt == 0), stop=(jt == 3))
                    h5 = sml.tile([LATENT + 1, 512], F16, tag="h5")
                    nc.gpsimd.memset(h5[:], 1.0)
                    nc.vector.tensor_scalar(h5[0:LATENT, :], preh[:],
                                            b1s[:], 0.0,
                                            mybir.AluOpType.add,
                                            mybir.AluOpType.max)
                    wnat = woutp.tile([128, 4 * INPUT_DIM], F32, tag="wnat")
                    for c in range(4):
                        nc.tensor.matmul(
                            wnat[:, 8 * c:8 * (c + 1)],
                            h5[:, 128 * c:128 * (c + 1)], w2s[:],
                            start=True, stop=True)
                    nc.vector.tensor_copy(
                        onat[:, 32 * gg:32 * (gg + 1)], wnat[:])
                nc.scalar.dma_start(
                    out.rearrange("(p n) d -> p n d", n=64)[:, c0:c0 + nch, :],
                    onat[:])

            PHASES = [(0, 32), (32, 32)]
            mts = [stage_q(*PHASES[0]), stage_q(*PHASES[1])]
            for i, ph in enumerate(PHASES):
                if i + 2 < len(PHASES):
                    mts.append(stage_q(*PHASES[i + 2]))
                compute_q(*ph, *mts[i])

    nc.compile()
    return nc


_NC_CACHE = []


def _get_nc():
    if not _NC_CACHE:
        _NC_CACHE.append(_build_nc())
    return _NC_CACHE[0]


def kernel(x, q_weights, w1, b1, w2, b2):
    global LAST_RESULTS
    x = np.ascontiguousarray(np.asarray(x, dtype=np.float32))
    consts = _host_consts(np.asarray(q_weights), np.asarray(w1),
                          np.asarray(b1), np.asarray(w2), np.asarray(b2))
    nc = _get_nc()
    in_maps = [
        {'xs': np.ascontiguousarray(x[i * BC:(i + 1) * BC]), **consts}
        for i in range(NCORES)
    ]
    res = run_bass_kernel_spmd(nc, in_maps, list(range(NCORES)))
    LAST_RESULTS = res
    return np.concatenate([res.results[i]['out'] for i in range(NCORES)],
                          axis=0).astype(np.float32)


# revision 28
# speedup vs baseline: 1.1622x; 1.0291x over previous
"""Trainium2 Bass kernel for nn_AutoencoderHybrid_65481071408310.

Math: the reference simulates an 8-qubit circuit per sample. The RX-encoding
layer produces a product state whose amplitudes factor as
    psi[k] = m[k] * (-i)^popcount(k),   m[k] = prod_i (cos(x_i/2) or sin(x_i/2))
and the StronglyEntanglingLayers form a fixed 256x256 unitary U that depends
only on q_weights.  Folding the popcount phases into U gives a REAL matmul
    phi = m @ V,  V = [Re(W) | Im(W)],  W = (U * (-i)^popcount)^T   (256 x 512)
then probs = phi_r^2 + phi_i^2, z_i = probs @ signs, and the MLP head.
signs@w1.T folds into A (256x4); stacking A2=[A;A] lets the squared 512-wide
phi contract directly (no pairwise adds).

Device pipeline per core (batch 8192, fp16 matmul operands):
  ACT: cos/sin; PE: transpose to (wire, sample) layout; replication DMAs +
  DVE/GPSIMD fp16 muls build the outer-product mT (256 x samples) in
  transposed layout; PE: K=256 matmul -> phi (512 wide), squares (ACT+DVE),
  PE: A2 contraction (K=512 -> 4), relu (+b1) on ACT, PE: w2 head (+b2 on
  copy-out), strided DMA to (B, 8).
"""
import sys
import numpy as np

sys.path.insert(0, '/opt/trn_rl_repo')

import concourse.bacc as bacc
import concourse.mybir as mybir
import concourse.tile as tile
from concourse.bass_utils import run_bass_kernel_spmd

F32 = mybir.dt.float32
F16 = mybir.dt.float16
AFT = mybir.ActivationFunctionType
ALU = mybir.AluOpType

NQ = 8
DIM = 256
REPS = 4
INPUT_DIM = 8
LATENT = 4
BATCH = 65536
NCORES = 8
BC = BATCH // NCORES          # 8192 samples per core
NCHUNK = BC // 128            # 64 chunks of 128 samples
NCTILE = NCHUNK // 16         # 4 ctile groups (16 chunks each)
CF = 16 * 128                 # 2048 free elems per ctile
NBLK = BC // 512              # 16 blocks of 512 samples
BPC = 4                       # blocks per ctile

LAST_RESULTS = None           # test harness introspection


# ---------------------------------------------------------------- host math
def _rot_mat(phi, theta, omega):
    c, s = np.cos(theta / 2), np.sin(theta / 2)
    return np.array([
        [np.exp(-0.5j * (phi + omega)) * c, -np.exp(0.5j * (phi - omega)) * s],
        [np.exp(-0.5j * (phi - omega)) * s, np.exp(0.5j * (phi + omega)) * c],
    ], dtype=np.complex128)


def _kron_list(ops):
    full = ops[0]
    for o in ops[1:]:
        full = np.kron(full, o)
    return full


def _build_entangler(qw):
    I2 = np.eye(2, dtype=np.complex128)
    P0 = np.array([[1, 0], [0, 0]], dtype=np.complex128)
    P1 = np.array([[0, 0], [0, 1]], dtype=np.complex128)
    X = np.array([[0, 1], [1, 0]], dtype=np.complex128)
    U = np.eye(DIM, dtype=np.complex128)
    for l in range(REPS):
        for i in range(NQ):
            ops = [I2] * NQ
            ops[i] = _rot_mat(*qw[l, i])
            U = _kron_list(ops) @ U
        r = (l % (NQ - 1)) + 1
        for i in range(NQ):
            t = (i + r) % NQ
            ops0 = [I2] * NQ
            ops0[i] = P0
            ops1 = [I2] * NQ
            ops1[i] = P1
            ops1[t] = X
            U = (_kron_list(ops0) + _kron_list(ops1)) @ U
    return U


def _host_consts(q_weights, w1, b1, w2, b2):
    U = _build_entangler(q_weights.astype(np.float64))
    pop = np.array([bin(k).count('1') for k in range(DIM)])
    W = (U * ((-1j) ** pop)[None, :]).T          # phi = m @ W
    V = np.concatenate([W.real, W.imag], axis=1)  # (256, 512)
    ks = np.arange(DIM)
    signs = 1.0 - 2.0 * ((ks[:, None] >> (NQ - 1 - np.arange(NQ))[None, :]) & 1)
    A = signs @ w1.T.astype(np.float64)           # (256, 4)
    vmat = np.ascontiguousarray(
        V.reshape(2, 128, 512).transpose(1, 0, 2).reshape(128, 1024)
        .astype(np.float16))
    amat = np.ascontiguousarray(
        A.reshape(2, 128, LATENT).transpose(1, 0, 2).reshape(128, 2 * LATENT)
        .astype(np.float16))
    w2b = np.concatenate([w2.T.astype(np.float64),
                          b2.astype(np.float64)[None, :]], axis=0)  # (5, 8)
    return {
        'vmat': vmat,
        'amat': amat,
        'w2b': np.ascontiguousarray(w2b.astype(np.float16)),
        'b1c': np.ascontiguousarray(b1.astype(np.float32).reshape(LATENT, 1)),
        'ident': np.eye(128, dtype=np.float16),
    }


# ---------------------------------------------------------------- bass build
def _build_nc():
    nc = bacc.Bacc(None, target_bir_lowering=False)
    xs = nc.declare_dram_parameter("xs", [BC, INPUT_DIM], F32, isOutput=False)
    vmat = nc.declare_dram_parameter("vmat", [128, 1024], F16, isOutput=False)
    amat = nc.declare_dram_parameter("amat", [128, 2 * LATENT], F16, isOutput=False)
    w2b = nc.declare_dram_parameter("w2b", [LATENT + 1, INPUT_DIM], F16, isOutput=False)
    b1c = nc.declare_dram_parameter("b1c", [LATENT, 1], F32, isOutput=False)
    ident = nc.declare_dram_parameter("ident", [128, 128], F16, isOutput=False)
    out = nc.declare_dram_parameter("out", [BC, INPUT_DIM], F32, isOutput=True)

    CH = 4096              # free elems per half (32 chunks)

    with tile.TileContext(nc) as tc:
        with (
            tc.tile_pool(name="const", bufs=1) as cst,
            tc.tile_pool(name="cs", bufs=1) as csp,
            tc.tile_pool(name="stage", bufs=1) as stg,
            tc.tile_pool(name="mtp", bufs=2) as mtp,
            tc.tile_pool(name="blk", bufs=6) as blk,
            tc.tile_pool(name="small", bufs=2) as sml,
        ):
            # ---- input load first (critical path)
            xnat = csp.tile([128, BC // 16], F32)      # free = (n, d)
            nc.sync.dma_start(xnat[:], xs.rearrange("(p n) d -> p n d", n=64))
            # ---- constants
            vt = cst.tile([128, 1024], F16)
            nc.sync.dma_start(vt[:], vmat[:])
            at = cst.tile([128, 2 * LATENT], F16)
            nc.sync.dma_start(at[:], amat[:])
            w2s = cst.tile([LATENT + 1, INPUT_DIM], F16)
            nc.sync.dma_start(w2s[:], w2b[:])
            b1s = cst.tile([LATENT, 1], F32)
            nc.sync.dma_start(b1s[:], b1c[:])
            ids = cst.tile([128, 128], F16)
            nc.sync.dma_start(ids[:], ident[:])
            halfpi = cst.tile([128, 1], F32)
            nc.vector.memset(halfpi[:], float(np.pi / 2))
            zero = cst.tile([128, 1], F32)
            nc.vector.memset(zero[:], 0.0)

            # ---- whole-core cos/sin, natural layout; sample = 64p + n
            # prime the Sin table before x arrives
            warm = cst.tile([1, 1], F16)
            nc.scalar.activation(warm[:], zero[0:1, :], AFT.Sin, scale=1.0,
                                 bias=zero[0:1, :])
            cnat = csp.tile([128, BC // 16], F16)
            snat = csp.tile([128, BC // 16], F16)
            xdn = xnat.rearrange("p (n d) -> p d n", d=8)
            nc.scalar.activation(cnat.rearrange("p (d n) -> p d n", d=8),
                                 xdn, AFT.Sin, scale=0.5, bias=halfpi[:])
            nc.scalar.activation(snat.rearrange("p (d n) -> p d n", d=8),
                                 xdn, AFT.Sin, scale=0.5, bias=zero[:])

            # ---- all 8 transposes upfront into per-ctile (wire, sample) tiles
            # cnat free = (d, n): slice u holds wires {2u, 2u+1} x n in [0,64)
            # cTs[u]: row 64*(w%2)+n = wire w=2u+(w%2), chunk n
            cTs, sTs = [], []
            with tc.tile_pool(name="tps", bufs=1, space="PSUM") as tpsp:
              for u in range(4):
                ctp = tpsp.tile([128, 128], F16, tag="tp")
                nc.tensor.transpose(ctp[:], cnat[:, 128 * u:128 * (u + 1)], ids[:])
                cTu = csp.tile([128, 128], F16, tag=f"cT{u}")
                nc.vector.tensor_copy(cTu[:], ctp[:])
                cTs.append(cTu)
                stp = tpsp.tile([128, 128], F16, tag="tp")
                nc.tensor.transpose(stp[:], snat[:, 128 * u:128 * (u + 1)], ids[:])
                sTu = csp.tile([128, 128], F16, tag=f"sT{u}")
                nc.vector.tensor_copy(sTu[:], stp[:])
                sTs.append(sTu)

            def stage_q(c0, nch):
                CH = 128 * nch
                csf = stg.tile([16, CH], F16, tag="csf")
                for w in range(8):
                    rows = slice(64 * (w % 2) + c0, 64 * (w % 2) + c0 + nch)
                    nc.sync.dma_start(csf[w:w + 1, :], cTs[w // 2][rows, :])
                    nc.scalar.dma_start(csf[8 + w:9 + w, :], sTs[w // 2][rows, :])

                pairsA = stg.tile([16, CH], F16, tag="pairsA")
                pairsB = stg.tile([16, CH], F16, tag="pairsB")
                for q in range(4):
                    nc.gpsimd.dma_start(
                        pairsA[4 * q:4 * q + 4, :],
                        csf[2 * q::8, :].unsqueeze(1).broadcast_to([2, 2, CH]))
                    nc.sync.dma_start(pairsB[4 * q:4 * q + 2, :],
                                      csf[2 * q + 1::8, :])
                    nc.sync.dma_start(pairsB[4 * q + 2:4 * q + 4, :],
                                      csf[2 * q + 1::8, :])
                pairs = stg.tile([16, CH], F16, tag="pairs")
                nc.vector.tensor_mul(pairs[:], pairsA[:], pairsB[:])

                hiloA = stg.tile([32, CH], F16, tag="hiloA")
                hiloB = stg.tile([32, CH], F16, tag="hiloB")
                nc.gpsimd.dma_start(
                    hiloA[0:16], pairs[0:4].unsqueeze(1).broadcast_to([4, 4, CH]))
                nc.gpsimd.dma_start(
                    hiloA[16:32], pairs[8:12].unsqueeze(1).broadcast_to([4, 4, CH]))
                for k in range(4):
                    nc.sync.dma_start(hiloB[4 * k:4 * k + 4], pairs[4:8])
                    nc.sync.dma_start(hiloB[16 + 4 * k:20 + 4 * k], pairs[12:16])
                hilo = stg.tile([32, CH], F16, tag="hilo")
                nc.vector.tensor_mul(hilo[:], hiloA[:], hiloB[:])

                mtA0 = stg.tile([128, CH], F16, tag="mtA0")
                mtA1 = stg.tile([128, CH], F16, tag="mtA1")
                mtB = stg.tile([128, CH], F16, tag="mtB")
                h4 = stg.tile([96, CH], F16, tag="h4")
                nc.gpsimd.dma_start(
                    h4[0:32], hilo[0:8].unsqueeze(1).broadcast_to([8, 4, CH]))
                nc.gpsimd.dma_start(
                    h4[32:64], hilo[8:16].unsqueeze(1).broadcast_to([8, 4, CH]))
                nc.sync.dma_start(h4[64:80], hilo[16:32])
                nc.sync.dma_start(h4[80:96], hilo[16:32])
                nc.gpsimd.dma_start(
                    mtA0[:], h4[0:32].unsqueeze(1).broadcast_to([32, 4, CH]))
                nc.gpsimd.dma_start(
                    mtA1[:], h4[32:64].unsqueeze(1).broadcast_to([32, 4, CH]))
                nc.gpsimd.dma_start(mtB[0:32], h4[64:96])
                nc.gpsimd.dma_start(mtB[32:64], h4[64:96])
                nc.sync.dma_start(mtB[64:96], h4[64:96])
                nc.sync.dma_start(mtB[96:128], h4[64:96])
                mt0 = mtp.tile([128, CH], F16, tag="mt0")
                mt1 = mtp.tile([128, CH], F16, tag="mt1")
                nc.vector.tensor_mul(mt0[:], mtA0[:], mtB[:])
                nc.vector.tensor_mul(mt1[:], mtA1[:], mtB[:])
                return mt0, mt1

            pools = {}

            def compute_q(c0, nch, mt0, mt1):
                phip = pools['phip']
                prehp = pools['prehp']
                woutp = pools['woutp']
                nblk = nch // 4
                onat = sml.tile([128, 8 * nch], F32, tag="onat")
                for gg in range(nblk):
                    sl = slice(512 * gg, 512 * (gg + 1))
                    probs = []
                    for jp in range(2):
                        phi = phip.tile([128, 1024], F32, tag="phi")
                        for e in range(2):
                            jt = 2 * jp + e
                            nc.tensor.matmul(
                                phi[:, 512 * e:512 * (e + 1)],
                                vt[:, 128 * jt:128 * (jt + 1)],
                                mt0[:, sl], start=True, stop=False)
                            nc.tensor.matmul(
                                phi[:, 512 * e:512 * (e + 1)],
                                vt[:, 512 + 128 * jt:512 + 128 * (jt + 1)],
                                mt1[:, sl], start=False, stop=True)
                        pr = blk.tile([128, 1024], F16, tag="probs")
                        nc.scalar.activation(pr[:], phi[:], AFT.Square,
                                             bias=zero[:])
                        probs.append(pr)
                    preh = prehp.tile([LATENT, 512], F32, tag="preh")
                    for jt in range(4):
                        ab = at[:, 4 * (jt % 2):4 * (jt % 2) + 4]
                        nc.tensor.matmul(preh[:],
                                         ab, probs[jt // 2][:, 512 * (jt % 2):
                                                            512 * (jt % 2) + 512],
                                         start=(jt == 0), stop=(jt == 3))
                    h5 = sml.tile([LATENT + 1, 512], F16, tag="h5")
                    nc.gpsimd.memset(h5[:], 1.0)
                    nc.vector.tensor_scalar(h5[0:LATENT, :], preh[:],
                                            b1s[:], 0.0,
                                            mybir.AluOpType.add,
                                            mybir.AluOpType.max)
                    wnat = woutp.tile([128, 4 * INPUT_DIM], F32, tag="wnat")
                    for c in range(4):
                        nc.tensor.matmul(
                            wnat[:, 8 * c:8 * (c + 1)],
                            h5[:, 128 * c:128 * (c + 1)], w2s[:],
                            start=True, stop=True)
                    nc.vector.tensor_copy(
                        onat[:, 32 * gg:32 * (gg + 1)], wnat[:])
                nc.scalar.dma_start(
                    out.rearrange("(p n) d -> p n d", n=64)[:, c0:c0 + nch, :],
                    onat[:])

            PHASES = [(0, 32), (32, 32)]
            with (
                tc.tile_pool(name="phip", bufs=3, space="PSUM") as phip_,
                tc.tile_pool(name="prehp", bufs=1, space="PSUM") as prehp_,
                tc.tile_pool(name="woutp", bufs=1, space="PSUM") as woutp_,
            ):
                pools['phip'] = phip_
                pools['prehp'] = prehp_
                pools['woutp'] = woutp_
                mts = [stage_q(*PHASES[0]), stage_q(*PHASES[1])]
                for i, ph in enumerate(PHASES):
                    if i + 2 < len(PHASES):
                        mts.append(stage_q(*PHASES[i + 2]))
                    compute_q(*ph, *mts[i])

    nc.compile()
    return nc


_NC_CACHE = []


def _get_nc():
    if not _NC_CACHE:
        _NC_CACHE.append(_build_nc())
    return _NC_CACHE[0]


def kernel(x, q_weights, w1, b1, w2, b2):
    global LAST_RESULTS
    x = np.ascontiguousarray(np.asarray(x, dtype=np.float32))
    consts = _host_consts(np.asarray(q_weights), np.asarray(w1),
                          np.asarray(b1), np.asarray(w2), np.asarray(b2))
    nc = _get_nc()
    in_maps = [
        {'xs': np.ascontiguousarray(x[i * BC:(i + 1) * BC]), **consts}
        for i in range(NCORES)
    ]
    res = run_bass_kernel_spmd(nc, in_maps, list(range(NCORES)))
    LAST_RESULTS = res
    return np.concatenate([res.results[i]['out'] for i in range(NCORES)],
                          axis=0).astype(np.float32)
